# revision 30
# baseline (speedup 1.0000x reference)
"""Trainium2 Bass kernel for LocallyConnectedLinear.

Problem: out[b,h,w,o] = sum_k x_unf[b,h,w,k] * lc_params[h,w,k,o]
  x: [8,32,32,64] f32, lc_params: [30,30,576,64] f32 (k = c*9 + i*3 + j),
  out: [8,30,30,64] f32.

The weight tensor (132.7 MB, used once at 8 MACs/element) dominates; the
kernel sits at the fp32 roofline ridge: weight DMA ~67us/core vs fp32 PE
streaming ~78us/core, overlapped. Strategy:
  - Shard the 900 (h,w) output locations across the 8 cores (113/112 each,
    padded to 120 = 15 groups of 8). Each core streams only its weight slice;
    no collectives needed (disjoint outputs).
  - Host-side im2col (a pure layout transform, same byte count as x itself)
    produces a K-major patch matrix xT [576, 960] per core (m = loc*8 + b).
  - Weights are host-transposed to [K, loc, O] so each SBUF partition (one
    k-row) receives a fully contiguous (loc*o) run -> 6KB DMA descriptors
    (vs 256B in [loc, K, O] order, which capped DMA at ~45% of HBM rate).
  - Per group g of 8 locations: lhsT = xT[:, 64g:64g+64] (stationary,
    [K,64] = 8 locs x 8 batch), rhs = weights [K, 512] (8 locs x 64 outs),
    accumulated over 5 K-chunks into one PSUM bank [64, 512]. The useful
    outputs are the 8 diagonal [8b, 64o] blocks; the full [64,512] tile is
    copied out and the diagonal extracted on host (one dense DVE copy beats
    8 sparse on-chip copies).
  - Strict fp32 matmuls (rel err ~6e-7). float32r would cut PE time 4x but
    costs ~1.5e-4 rel err; flip _build_nc(use_f32r=True) if tolerance allows.

Measured on 8 axon-tunneled trn2 cores: ~100-120us HW exec per core
(varies with chip power state), rel err 6.2e-07.
"""

import json
import sys

for _p in ("/opt/trn_rl_repo/concourse", "/opt/trn_rl_repo"):
    if _p not in sys.path:
        sys.path.insert(0, _p)

import numpy as np

import concourse.bass as bass
import concourse.mybir as mybir
import concourse.tile as tile
from concourse.bass_utils import run_bass_kernel_spmd

N_CORES = 8
B = 8
H_OUT = W_OUT = 30
N_LOC = H_OUT * W_OUT  # 900
K = 576
O = 64
LOCS_PER_CORE = 113  # 4 cores hold 113 real locs, 4 hold 112 (pad 1)
GROUP = 8  # locations per matmul group (N = 8*64 = 512)
N_FULL = 14  # full groups of 8; one tail group of 1 loc (N = 64)
N_GROUPS = 15  # output slots (14 full + tail)
K_CHUNKS = [(0, 128), (128, 128), (256, 128), (384, 128), (512, 64)]

# ---------------------------------------------------------------------------
# BIR post-pass: this walrus build accepts at most ONE sync wait per
# instruction (seen on Drain/TPB_CTRL and Matmult/S3_LW encodings), but Tile
# emits instructions carrying several waits (e.g. the kernel-tail drain
# gathers every dangling DMA semaphore). Hoist excess waits onto inserted
# single-wait EventSemaphore instructions immediately before the offender
# (same engine => same sequencer program order => identical semantics).
# ---------------------------------------------------------------------------
_WAIT_CAP = 1
_uid = [0]


def _hoist_inst(engine, wait, debug):
    _uid[0] += 1
    return {
        "debug": debug,
        "engine": engine,
        "ins": [],
        "name": f"hoistw-{_uid[0]}",
        "opcode": "EventSemaphore",
        "outs": [],
        "sync_info": {"on_update": [], "on_wait": [wait]},
    }


def _fix_bir_json(data: bytes) -> bytes:
    bir = json.loads(data)
    changed = False
    for fn in bir.get("functions", []):
        for blk in fn.get("blocks", []):
            out = []
            for ins in blk.get("instructions", []):
                si = ins.get("sync_info") or {}
                waits = si.get("on_wait") or []
                if len(waits) > _WAIT_CAP:
                    for w in waits[:-_WAIT_CAP]:
                        out.append(_hoist_inst(ins["engine"], w, ins.get("debug")))
                    si["on_wait"] = waits[-_WAIT_CAP:]
                    ins["sync_info"] = si
                    changed = True
                out.append(ins)
            blk["instructions"] = out
    return json.dumps(bir).encode() if changed else data


def _install_birfix(nc):
    orig = nc.to_json_bytes

    def patched(*a, **k):
        return _fix_bir_json(orig(*a, **k))

    nc.to_json_bytes = patched


# ---------------------------------------------------------------------------
# Bass kernel (identical NEFF on all 8 cores; per-core data differs)
# ---------------------------------------------------------------------------
BLOCK_GROUPS = [3, 3, 3, 3, 2]  # 14 full groups; tail loc handled after


def _build_nc(use_f32r=False):
    f32 = mybir.dt.float32
    wdt = mybir.dt.float32r if use_f32r else f32
    M = LOCS_PER_CORE * B  # 904
    nc = bass.Bass()
    xT = nc.dram_tensor("xT", [K, M], wdt, kind="ExternalInput")
    # weights pre-transposed on host to [K, loc, O]: each SBUF partition (one
    # k-row) receives a fully contiguous (loc, o) run -> multi-KB descriptors
    wT = nc.dram_tensor("wT", [K, LOCS_PER_CORE, O], wdt, kind="ExternalInput")
    out = nc.dram_tensor("out", [N_GROUPS, GROUP * B, GROUP * O], f32,
                         kind="ExternalOutput")

    with tile.TileContext(nc) as tc:
        with (
            tc.tile_pool(name="xpool", bufs=1) as xpool,
            tc.tile_pool(name="wpool", bufs=12) as wpool,
            tc.tile_pool(name="psum", bufs=8, space="PSUM") as psum,
            tc.tile_pool(name="stage", bufs=8) as stage,
        ):
            # interleave the x-chunk loads with block-0 weight chunks so the
            # first matmul's inputs (x chunk 0, w block-0 chunk 0) land first
            xq = []
            wts0 = []
            bl_first = BLOCK_GROUPS[0] * GROUP
            for ci, (q0, kq) in enumerate(K_CHUNKS):
                wt = wpool.tile([kq, 3 * GROUP, O], wdt, tag="wt")
                nc.sync.dma_start(
                    out=wt[:, :bl_first, :], in_=wT[q0 : q0 + kq, 0:bl_first, :]
                )
                wts0.append(wt)
                xt = xpool.tile([kq, M], wdt, tag=f"x{q0}")
                nc.scalar.dma_start(out=xt, in_=xT[q0 : q0 + kq, :])
                xq.append(xt)

            g_base = 0
            for blk, gb in enumerate(BLOCK_GROUPS):
                bl0, bln = g_base * GROUP, gb * GROUP
                if blk == 0:
                    wts = wts0
                else:
                    wts = []
                    for ci, (q0, kq) in enumerate(K_CHUNKS):
                        ring = nc.sync if (blk * 5 + ci) % 2 == 0 else nc.scalar
                        wt = wpool.tile([kq, 3 * GROUP, O], wdt, tag="wt")
                        ring.dma_start(
                            out=wt[:, :bln, :],
                            in_=wT[q0 : q0 + kq, bl0 : bl0 + bln, :],
                        )
                        wts.append(wt)
                for gi in range(gb):
                    g = g_base + gi
                    lo, hi = g * GROUP, (g + 1) * GROUP
                    ps = psum.tile([GROUP * B, GROUP * O], f32, tag="ps")
                    for ci in range(len(K_CHUNKS)):
                        nc.tensor.matmul(
                            ps,
                            xq[ci][:, lo * B : hi * B],
                            wts[ci][:, gi * GROUP : (gi + 1) * GROUP, :],
                            start=(ci == 0),
                            stop=(ci == len(K_CHUNKS) - 1),
                        )
                    st = stage.tile([GROUP * B, GROUP * O], f32, tag="st")
                    nc.vector.tensor_copy(st, ps)
                    engine = nc.scalar if gi % 2 == 0 else nc.sync
                    engine.dma_start(out=out[g], in_=st)
                g_base += gb

            # tail group: one location (N = 64)
            t0 = N_FULL * GROUP  # loc 112
            wtl = []
            for ci, (q0, kq) in enumerate(K_CHUNKS):
                wt = wpool.tile([kq, 1, O], wdt, tag="wtail")
                nc.scalar.dma_start(out=wt, in_=wT[q0 : q0 + kq, t0 : t0 + 1, :])
                wtl.append(wt)
            ps = psum.tile([B, O], f32, tag="ps")
            for ci in range(len(K_CHUNKS)):
                nc.tensor.matmul(
                    ps,
                    xq[ci][:, t0 * B : (t0 + 1) * B],
                    wtl[ci][:, 0, :],
                    start=(ci == 0),
                    stop=(ci == len(K_CHUNKS) - 1),
                )
            st = stage.tile([B, O], f32, tag="st")
            nc.vector.tensor_copy(st, ps)
            nc.sync.dma_start(out=out[N_FULL, :B, :O], in_=st)

    _install_birfix(nc)
    return nc


# ---------------------------------------------------------------------------
# Host wrapper
# ---------------------------------------------------------------------------
def _core_splits():
    counts = [113] * 4 + [112] * 4  # sums to 900
    starts = np.cumsum([0] + counts[:-1]).tolist()
    return list(zip(starts, counts))


def _axon_devices_ok() -> bool:
    try:
        import jax

        return len(jax.devices()) >= N_CORES
    except Exception:
        return False


def _kernel_via_subprocess(x: np.ndarray, lc_params: np.ndarray) -> np.ndarray:
    """Fallback when the calling process pinned jax to a non-axon platform
    (the backend caches cannot be repointed reliably in-process): run the
    kernel in a clean child interpreter where jax discovers the 8 cores."""
    import os
    import subprocess
    import tempfile

    here = os.path.dirname(os.path.abspath(__file__))
    with tempfile.TemporaryDirectory() as td:
        np.savez(os.path.join(td, "in.npz"), x=x, lc_params=lc_params)
        env = dict(os.environ)
        env.pop("JAX_PLATFORMS", None)
        env["_LCL_KERNEL_CHILD"] = "1"
        code = (
            "import sys, numpy as np\n"
            f"sys.path.insert(0, {here!r})\n"
            "from kernel import kernel\n"
            f"d = np.load({os.path.join(td, 'in.npz')!r})\n"
            "out = kernel(d['x'], d['lc_params'])\n"
            f"np.save({os.path.join(td, 'out.npy')!r}, out)\n"
        )
        subprocess.run([sys.executable, "-c", code], env=env, check=True)
        return np.load(os.path.join(td, "out.npy"))


def kernel(x: np.ndarray, lc_params: np.ndarray) -> np.ndarray:
    if not _axon_devices_ok():
        import os

        if os.environ.get("_LCL_KERNEL_CHILD"):
            raise RuntimeError("no axon neuron cores visible to jax")
        return _kernel_via_subprocess(x, lc_params)
    x = np.ascontiguousarray(x, dtype=np.float32)
    lc = np.ascontiguousarray(lc_params, dtype=np.float32).reshape(N_LOC, K, O)

    # im2col, feature order (c, kh, kw) with c slowest: k = c*9 + i*3 + j
    sw = np.lib.stride_tricks.sliding_window_view(x, (3, 3), axis=(1, 2))
    x_unf = sw.reshape(B, N_LOC, K)  # [b, loc, k]

    in_maps = []
    for s, n in _core_splits():
        xTc = np.zeros((K, LOCS_PER_CORE * B), dtype=np.float32)
        # m = loc*8 + b  (b fastest)
        xTc[:, : n * B] = (
            x_unf[:, s : s + n, :].transpose(2, 1, 0).reshape(K, n * B)
        )
        wc = np.zeros((K, LOCS_PER_CORE, O), dtype=np.float32)
        wc[:, :n, :] = lc[s : s + n].transpose(1, 0, 2)
        in_maps.append({"xT": xTc, "wT": wc})

    nc = _build_nc()
    res = run_bass_kernel_spmd(nc, in_maps, core_ids=list(range(N_CORES)))

    out = np.empty((B, N_LOC, O), dtype=np.float32)
    idx = np.arange(GROUP)
    for core, (s, n) in enumerate(_core_splits()):
        r = res.results[core]["out"].reshape(N_GROUPS, GROUP, B, GROUP, O)
        # diagonal l == l2 -> [l, g, b, o] -> [b, g, l, o] -> [b, loc, o]
        d = r[:, idx, :, idx, :].transpose(2, 1, 0, 3).reshape(B, N_GROUPS * GROUP, O)
        out[:, s : s + n, :] = d[:, :n, :]

    return out.reshape(B, H_OUT, W_OUT, O)


if __name__ == "__main__":
    xs = np.random.randn(B, 32, 32, 64).astype(np.float32)
    ws = (np.random.randn(H_OUT, W_OUT, K, O) * 0.02).astype(np.float32)
    r = kernel(xs, ws)
    print("kernel output shape:", r.shape)


# revision 32
# speedup vs baseline: 1.0869x; 1.0869x over previous
"""Trainium2 Bass kernel for LocallyConnectedLinear.

Problem: out[b,h,w,o] = sum_k x_unf[b,h,w,k] * lc_params[h,w,k,o]
  x: [8,32,32,64] f32, lc_params: [30,30,576,64] f32 (k = c*9 + i*3 + j),
  out: [8,30,30,64] f32.

The weight tensor (132.7 MB, used once at 8 MACs/element) dominates; the
kernel sits at the fp32 roofline ridge: weight DMA ~67us/core vs fp32 PE
streaming ~78us/core, overlapped. Strategy:
  - Shard the 900 (h,w) output locations across the 8 cores (113/112 each,
    padded to 113 = 14 groups of 8 + a 1-loc tail group). Each core streams
    only its weight slice; no collectives needed (disjoint outputs).
  - Host-side im2col (a pure layout transform, same byte count as x itself)
    produces a K-major patch matrix xT [576, 904] per core (m = loc*8 + b).
  - Weights are host-transposed to [K, loc, O] so each SBUF partition (one
    k-row) receives a fully contiguous (loc*o) run -> 6KB DMA descriptors
    (vs 256B in [loc, K, O] order, which capped DMA at ~45% of HBM rate).
  - Per group g of 8 locations: lhsT = xT[:, 64g:64g+64] (stationary,
    [K,64] = 8 locs x 8 batch), rhs = weights [K, 512] (8 locs x 64 outs),
    accumulated over 5 K-chunks into one PSUM bank [64, 512]. The useful
    outputs are the 8 diagonal [8b, 64o] blocks; the full [64,512] tile is
    copied out and the diagonal extracted on host (one dense DVE copy beats
    8 sparse on-chip copies).
  - Strict fp32 matmuls (rel err ~6e-7). float32r would cut PE time 4x but
    costs ~1.5e-4 rel err; flip _build_nc(use_f32r=True) if tolerance allows.

Measured on 8 axon-tunneled trn2 cores: ~97-103us HW exec per core in the
chip's fast power state (up to ~110-120us when hot), rel err 6.2e-07.
"""

import json
import sys

for _p in ("/opt/trn_rl_repo/concourse", "/opt/trn_rl_repo"):
    if _p not in sys.path:
        sys.path.insert(0, _p)

import numpy as np

import concourse.bass as bass
import concourse.mybir as mybir
import concourse.tile as tile
from concourse.bass_utils import run_bass_kernel_spmd

N_CORES = 8
B = 8
H_OUT = W_OUT = 30
N_LOC = H_OUT * W_OUT  # 900
K = 576
O = 64
LOCS_PER_CORE = 113  # 4 cores hold 113 real locs, 4 hold 112 (pad 1)
GROUP = 8  # locations per matmul group (N = 8*64 = 512)
N_FULL = 14  # full groups of 8; one tail group of 1 loc (N = 64)
N_GROUPS = 15  # output slots (14 full + tail)
K_CHUNKS = [(0, 128), (128, 128), (256, 128), (384, 128), (512, 64)]

# ---------------------------------------------------------------------------
# BIR post-pass: this walrus build accepts at most ONE sync wait per
# instruction (seen on Drain/TPB_CTRL and Matmult/S3_LW encodings), but Tile
# emits instructions carrying several waits (e.g. the kernel-tail drain
# gathers every dangling DMA semaphore). Hoist excess waits onto inserted
# single-wait EventSemaphore instructions immediately before the offender
# (same engine => same sequencer program order => identical semantics).
# ---------------------------------------------------------------------------
_WAIT_CAP = 1
_uid = [0]


def _hoist_inst(engine, wait, debug):
    _uid[0] += 1
    return {
        "debug": debug,
        "engine": engine,
        "ins": [],
        "name": f"hoistw-{_uid[0]}",
        "opcode": "EventSemaphore",
        "outs": [],
        "sync_info": {"on_update": [], "on_wait": [wait]},
    }


def _fix_bir_json(data: bytes) -> bytes:
    bir = json.loads(data)
    changed = False
    for fn in bir.get("functions", []):
        for blk in fn.get("blocks", []):
            out = []
            for ins in blk.get("instructions", []):
                si = ins.get("sync_info") or {}
                waits = si.get("on_wait") or []
                if len(waits) > _WAIT_CAP:
                    for w in waits[:-_WAIT_CAP]:
                        out.append(_hoist_inst(ins["engine"], w, ins.get("debug")))
                    si["on_wait"] = waits[-_WAIT_CAP:]
                    ins["sync_info"] = si
                    changed = True
                out.append(ins)
            blk["instructions"] = out
    return json.dumps(bir).encode() if changed else data


def _install_birfix(nc):
    orig = nc.to_json_bytes

    def patched(*a, **k):
        return _fix_bir_json(orig(*a, **k))

    nc.to_json_bytes = patched


# ---------------------------------------------------------------------------
# Bass kernel (identical NEFF on all 8 cores; per-core data differs)
# ---------------------------------------------------------------------------
BLOCK_GROUPS = [3, 3, 3, 3, 2]  # 14 full groups; tail loc handled after


def _build_nc(use_f32r=False):
    f32 = mybir.dt.float32
    wdt = mybir.dt.float32r if use_f32r else f32
    M = LOCS_PER_CORE * B  # 904
    nc = bass.Bass()
    xT = nc.dram_tensor("xT", [K, M], wdt, kind="ExternalInput")
    # weights pre-transposed on host to [K, loc, O]: each SBUF partition (one
    # k-row) receives a fully contiguous (loc, o) run -> multi-KB descriptors
    wT = nc.dram_tensor("wT", [K, LOCS_PER_CORE, O], wdt, kind="ExternalInput")
    out = nc.dram_tensor("out", [N_GROUPS, GROUP * B, GROUP * O], f32,
                         kind="ExternalOutput")

    with tile.TileContext(nc) as tc:
        with (
            tc.tile_pool(name="xpool", bufs=1) as xpool,
            tc.tile_pool(name="wpool", bufs=18) as wpool,
            tc.tile_pool(name="psum", bufs=8, space="PSUM") as psum,
            tc.tile_pool(name="stage", bufs=8) as stage,
        ):
            # interleave the x-chunk loads with block-0 weight chunks so the
            # first matmul's inputs (x chunk 0, w block-0 chunk 0) land first
            xq = []
            wts0 = []
            bl_first = BLOCK_GROUPS[0] * GROUP
            for ci, (q0, kq) in enumerate(K_CHUNKS):
                wt = wpool.tile([kq, 3 * GROUP, O], wdt, tag="wt")
                nc.sync.dma_start(
                    out=wt[:, :bl_first, :], in_=wT[q0 : q0 + kq, 0:bl_first, :]
                )
                wts0.append(wt)
                xt = xpool.tile([kq, M], wdt, tag=f"x{q0}")
                nc.scalar.dma_start(out=xt, in_=xT[q0 : q0 + kq, :])
                xq.append(xt)

            g_base = 0
            for blk, gb in enumerate(BLOCK_GROUPS):
                bl0, bln = g_base * GROUP, gb * GROUP
                if blk == 0:
                    wts = wts0
                else:
                    wts = []
                    for ci, (q0, kq) in enumerate(K_CHUNKS):
                        ring = nc.sync if (blk * 5 + ci) % 2 == 0 else nc.scalar
                        wt = wpool.tile([kq, 3 * GROUP, O], wdt, tag="wt")
                        ring.dma_start(
                            out=wt[:, :bln, :],
                            in_=wT[q0 : q0 + kq, bl0 : bl0 + bln, :],
                        )
                        wts.append(wt)
                for gi in range(gb):
                    g = g_base + gi
                    lo, hi = g * GROUP, (g + 1) * GROUP
                    ps = psum.tile([GROUP * B, GROUP * O], f32, tag="ps")
                    for ci in range(len(K_CHUNKS)):
                        nc.tensor.matmul(
                            ps,
                            xq[ci][:, lo * B : hi * B],
                            wts[ci][:, gi * GROUP : (gi + 1) * GROUP, :],
                            start=(ci == 0),
                            stop=(ci == len(K_CHUNKS) - 1),
                        )
                    st = stage.tile([GROUP * B, GROUP * O], f32, tag="st")
                    nc.vector.tensor_copy(st, ps)
                    engine = nc.scalar if gi % 2 == 0 else nc.sync
                    engine.dma_start(out=out[g], in_=st)
                g_base += gb

            # tail group: one location (N = 64)
            t0 = N_FULL * GROUP  # loc 112
            wtl = []
            for ci, (q0, kq) in enumerate(K_CHUNKS):
                wt = wpool.tile([kq, 1, O], wdt, tag="wtail")
                nc.scalar.dma_start(out=wt, in_=wT[q0 : q0 + kq, t0 : t0 + 1, :])
                wtl.append(wt)
            ps = psum.tile([B, O], f32, tag="ps")
            for ci in range(len(K_CHUNKS)):
                nc.tensor.matmul(
                    ps,
                    xq[ci][:, t0 * B : (t0 + 1) * B],
                    wtl[ci][:, 0, :],
                    start=(ci == 0),
                    stop=(ci == len(K_CHUNKS) - 1),
                )
            st = stage.tile([B, O], f32, tag="st")
            nc.vector.tensor_copy(st, ps)
            nc.sync.dma_start(out=out[N_FULL, :B, :O], in_=st)

    _install_birfix(nc)
    return nc


# ---------------------------------------------------------------------------
# Host wrapper
# ---------------------------------------------------------------------------
def _core_splits():
    counts = [113] * 4 + [112] * 4  # sums to 900
    starts = np.cumsum([0] + counts[:-1]).tolist()
    return list(zip(starts, counts))


def _axon_devices_ok() -> bool:
    try:
        import jax

        return len(jax.devices()) >= N_CORES
    except Exception:
        return False


def _kernel_via_subprocess(x: np.ndarray, lc_params: np.ndarray) -> np.ndarray:
    """Fallback when the calling process pinned jax to a non-axon platform
    (the backend caches cannot be repointed reliably in-process): run the
    kernel in a clean child interpreter where jax discovers the 8 cores."""
    import os
    import subprocess
    import tempfile

    here = os.path.dirname(os.path.abspath(__file__))
    with tempfile.TemporaryDirectory() as td:
        np.savez(os.path.join(td, "in.npz"), x=x, lc_params=lc_params)
        env = dict(os.environ)
        env.pop("JAX_PLATFORMS", None)
        env["_LCL_KERNEL_CHILD"] = "1"
        code = (
            "import sys, numpy as np\n"
            f"sys.path.insert(0, {here!r})\n"
            "from kernel import kernel\n"
            f"d = np.load({os.path.join(td, 'in.npz')!r})\n"
            "out = kernel(d['x'], d['lc_params'])\n"
            f"np.save({os.path.join(td, 'out.npy')!r}, out)\n"
        )
        subprocess.run([sys.executable, "-c", code], env=env, check=True)
        return np.load(os.path.join(td, "out.npy"))


def kernel(x: np.ndarray, lc_params: np.ndarray) -> np.ndarray:
    if not _axon_devices_ok():
        import os

        if os.environ.get("_LCL_KERNEL_CHILD"):
            raise RuntimeError("no axon neuron cores visible to jax")
        return _kernel_via_subprocess(x, lc_params)
    x = np.ascontiguousarray(x, dtype=np.float32)
    lc = np.ascontiguousarray(lc_params, dtype=np.float32).reshape(N_LOC, K, O)

    # im2col, feature order (c, kh, kw) with c slowest: k = c*9 + i*3 + j
    sw = np.lib.stride_tricks.sliding_window_view(x, (3, 3), axis=(1, 2))
    x_unf = sw.reshape(B, N_LOC, K)  # [b, loc, k]

    in_maps = []
    for s, n in _core_splits():
        xTc = np.zeros((K, LOCS_PER_CORE * B), dtype=np.float32)
        # m = loc*8 + b  (b fastest)
        xTc[:, : n * B] = (
            x_unf[:, s : s + n, :].transpose(2, 1, 0).reshape(K, n * B)
        )
        wc = np.zeros((K, LOCS_PER_CORE, O), dtype=np.float32)
        wc[:, :n, :] = lc[s : s + n].transpose(1, 0, 2)
        in_maps.append({"xT": xTc, "wT": wc})

    nc = _build_nc()
    res = run_bass_kernel_spmd(nc, in_maps, core_ids=list(range(N_CORES)))

    out = np.empty((B, N_LOC, O), dtype=np.float32)
    idx = np.arange(GROUP)
    for core, (s, n) in enumerate(_core_splits()):
        r = res.results[core]["out"].reshape(N_GROUPS, GROUP, B, GROUP, O)
        # diagonal l == l2 -> [l, g, b, o] -> [b, g, l, o] -> [b, loc, o]
        d = r[:, idx, :, idx, :].transpose(2, 1, 0, 3).reshape(B, N_GROUPS * GROUP, O)
        out[:, s : s + n, :] = d[:, :n, :]

    return out.reshape(B, H_OUT, W_OUT, O)


if __name__ == "__main__":
    xs = np.random.randn(B, 32, 32, 64).astype(np.float32)
    ws = (np.random.randn(H_OUT, W_OUT, K, O) * 0.02).astype(np.float32)
    r = kernel(xs, ws)
    print("kernel output shape:", r.shape)
